# revision 1
# baseline (speedup 1.0000x reference)
"""Trainium2 Bass kernel for EnhancedCrossAttention.

Shapes (hardcoded): B=4, C=256, H=W=28, heads=8, head_dim=32.
Sharding: 8 cores = 4 batches x 2 head-groups (4 heads each core).
Each core computes its batch's QKV (its head-group's Q/K, full V re-sliced),
attention for 4 heads, and a partial out-projection (contracting only its
128 attention-output channels). Host sums the two partials per batch, adds
the folded bias, folds the two spatial halves, and reshapes.

Host-side algebraic folds (all exact):
  - pos_emb enters only via the QKV matmul: b_eff = b_qkv + w_qkv @ pos
  - 1/sqrt(dh) folded into Q weights+bias
  - V bias contributes attn_out += b_v (softmax weights sum to 1), so it is
    folded through w_out into a constant added on the host.

Matmul dtypes: QKV/scores/out-proj in float32r (PE full rate, rel err
~2e-4 at K=128); AV + denominator in bf16 (f32r forbids PE column-tiling
— s3d3_mm_valid_dst_partition — and the 4-head column packing needs it;
softmax normalization cancels most of the bf16 rounding: ~1e-4 end-to-end).
f32r operands must be produced by an engine that rounds-on-write (walrus
verifier contract; DMA-direct f32r crashes the device), so DMA'd tensors
pass through one DVE round-copy; exp writes P directly as bf16.
"""

import os
import numpy as np

EXPW = int(os.environ.get("EXPW", "2048"))
RUMP = os.environ.get("RUMP", "packed")  # packed | loop
QKDT = os.environ.get("QKDT", "f32r")  # f32r | bf16 (bf16 measured no better, slightly worse accuracy)

B, C, H, W = 4, 256, 28, 28
N = H * W            # 784
S = 2 * N            # 1568 tokens
NH = 8
DH = 32
GH = 4               # heads per group (per core)
GC = GH * DH         # 128 channels per group

CHUNKS = [(0, 512), (512, 1024), (1024, 1536), (1536, 1568)]
KTILES = [(i * 128, min(128, S - i * 128)) for i in range((S + 127) // 128)]

_cache = {}
LAST_RESULTS = None


def _build_nc(repeat=1):
    import concourse.mybir as mybir
    import concourse.tile as tile
    from concourse import bacc
    from contextlib import ExitStack

    f32 = mybir.dt.float32
    f32r = mybir.dt.float32r
    bf16 = mybir.dt.bfloat16
    Exp = mybir.ActivationFunctionType.Exp

    nc = bacc.Bacc("TRN2", target_bir_lowering=False, debug=False)

    x1_d = nc.dram_tensor("x1b", [C, N], f32, kind="ExternalInput")
    x2_d = nc.dram_tensor("x2b", [C, N], f32, kind="ExternalInput")
    wqk_d = nc.dram_tensor("wqkT", [C, 256], f32, kind="ExternalInput")
    wv_d = nc.dram_tensor("wvT", [C, GC], f32, kind="ExternalInput")
    wout_d = nc.dram_tensor("woutT", [GC, 256], f32, kind="ExternalInput")
    bqk_d = nc.dram_tensor("bqk", [2, 128, 1], f32, kind="ExternalInput")
    y_d = nc.dram_tensor("y", [S, C], f32, kind="ExternalOutput")

    with tile.TileContext(nc) as tc:
      for _rep in range(repeat):
        ctx = ExitStack()
        pp = ctx.enter_context(tc.tile_pool(name="persist", bufs=1))
        sb = ctx.enter_context(tc.tile_pool(name="work", bufs=3))
        psb = ctx.enter_context(tc.tile_pool(name="pwork", bufs=6))

        # raw (DMA'd, f32)
        xT0 = [pp.tile([128, S], f32, name=f"xT0_{i}", tag=f"xT0_{i}") for i in range(2)]
        wqk0 = [pp.tile([128, 256], f32, name=f"wqk0_{i}", tag=f"wqk0_{i}") for i in range(2)]
        wv0 = [pp.tile([128, GC], f32, name=f"wv0_{i}", tag=f"wv0_{i}") for i in range(2)]
        wout0 = pp.tile([128, 256], f32, name="wout0", tag="wout0")
        bq_sb = pp.tile([128, 1], f32, name="bq", tag="bq")
        bk_sb = pp.tile([128, 1], f32, name="bk", tag="bk")
        # rounded (engine-written, f32r)
        xT = [pp.tile([128, S], f32r, name=f"xT{i}", tag=f"xT{i}") for i in range(2)]
        wqk_sb = [pp.tile([128, 256], f32r, name=f"wqk{i}", tag=f"wqk{i}") for i in range(2)]
        wv_sb = [pp.tile([128, GC], f32r, name=f"wv{i}", tag=f"wv{i}") for i in range(2)]
        wout_sb = pp.tile([128, 256], f32r, name="wout", tag="wout")
        qk_dt = bf16 if QKDT == "bf16" else f32r
        QT = pp.tile([128, S], qk_dt, name="QT", tag="QT")
        KTt = pp.tile([128, S], qk_dt, name="KT", tag="KT")
        Vt = [pp.tile([128, GC], bf16, name=f"V{i}", tag=f"V{i}") for i in range(len(KTILES))]
        ones_col = pp.tile([128, 32], f32, name="onescol", tag="onescol")
        ones_kt = pp.tile([128, 1], bf16, name="oneskt", tag="oneskt")
        zc = pp.tile([128, 32], f32, name="zc", tag="zc")

        # ones via exp(0): exact 1.0 and pre-loads the ACT exp table early
        nc.vector.memset(zc[:], 0.0)
        nc.scalar.activation(ones_col[:], zc[:], Exp)
        nc.scalar.activation(ones_kt[:], zc[:, 0:1], Exp)

        # --- DMA inputs + round-copies to f32r ---
        x1v = x1_d[:].rearrange("(t p) f -> t p f", p=128)
        x2v = x2_d[:].rearrange("(t p) f -> t p f", p=128)
        wqkv = wqk_d[:].rearrange("(t p) f -> t p f", p=128)
        wvv = wv_d[:].rearrange("(t p) f -> t p f", p=128)
        for t in range(2):
            nc.sync.dma_start(xT0[t][:, 0:N], x1v[t])
            nc.sync.dma_start(xT0[t][:, N:S], x2v[t])
            nc.sync.dma_start(wqk0[t][:], wqkv[t])
            nc.sync.dma_start(wv0[t][:], wvv[t])
            nc.vector.tensor_copy(wqk_sb[t][:], wqk0[t][:])
            # split the round-copy so the first QKV matmuls unblock early
            nc.vector.tensor_copy(xT[t][:, 0:N], xT0[t][:, 0:N])
            nc.vector.tensor_copy(xT[t][:, N:S], xT0[t][:, N:S])
            nc.vector.tensor_copy(wv_sb[t][:], wv0[t][:])
        nc.sync.dma_start(wout0[:], wout_d[:])
        nc.vector.tensor_copy(wout_sb[:], wout0[:])
        nc.sync.dma_start(bq_sb[:], bqk_d[0])
        nc.sync.dma_start(bk_sb[:], bqk_d[1])

        # --- QKV phase ---
        with tc.tile_pool(name="qkvps", bufs=2, space="PSUM") as qps:
            # chunk-major: Q-c0 and K-c0 land first so scores start earliest
            for (c0, c1) in CHUNKS:
                for m, (bias_t, out_t) in enumerate([(bq_sb, QT), (bk_sb, KTt)]):
                    w = c1 - c0
                    ps = qps.tile([128, 512], f32, name="qk", tag="qk")
                    for kt in range(2):
                        nc.tensor.matmul(
                            ps[:, :w],
                            wqk_sb[kt][:, 128 * m:128 * m + 128],
                            xT[kt][:, c0:c1],
                            start=(kt == 0), stop=(kt == 1),
                        )
                    nc.vector.tensor_scalar_add(out_t[:, c0:c1], ps[:, :w], bias_t[:])
            for i, (o, sz) in enumerate(KTILES):
                ps = qps.tile([128, GC], f32, name="v", tag="v")
                for kt in range(2):
                    nc.tensor.matmul(
                        ps[:sz, :],
                        xT[kt][:, o:o + sz],
                        wv_sb[kt][:],
                        start=(kt == 0), stop=(kt == 1),
                    )
                nc.vector.tensor_copy(Vt[i][:sz, :], ps[:sz, 0:GC])

        # --- attention ---
        with (
            tc.tile_pool(name="stps", bufs=(1 if EXPW == 2048 else 2), space="PSUM") as stps,
            tc.tile_pool(name="avps", bufs=1, space="PSUM") as avps,
            tc.tile_pool(name="denps", bufs=1, space="PSUM") as dnps,
            tc.tile_pool(name="tailps", bufs=2, space="PSUM") as tailps,
        ):
            main_chunks = CHUNKS[:3] if (EXPW == 2048 and RUMP == "packed") else CHUNKS
            for (c0, c1) in main_chunks:
                w = c1 - c0
                attnT_ps = avps.tile([128, 512], f32, name="attnT", tag="attnT")
                den_ps = dnps.tile([128, 512], f32, name="den", tag="den")
                for i, (o, sz) in enumerate(KTILES):
                    start = (i == 0)
                    stop = (i == len(KTILES) - 1)
                    if EXPW == 2048:
                        # one 4-bank st tile + a single wide exp per k-tile
                        st = stps.tile([128, 2048], f32, name="st", tag="st")
                        for h in range(4):
                            nc.tensor.matmul(
                                st[:sz, 512 * h:512 * h + w],
                                KTt[32 * h:32 * h + 32, o:o + sz],
                                QT[32 * h:32 * h + 32, c0:c1],
                                start=True, stop=True,
                                tile_position=(32 * h, 0),
                            )
                        P = psb.tile([128, 2048], bf16, name="P", tag="P")
                        if w == 512:
                            nc.scalar.activation(P[:sz, :], st[:sz, :], Exp)
                        else:
                            # rump chunk: one strided op over the four 32-wide blocks
                            stv = st[:sz, :].rearrange("p (h c) -> p h c", h=4)[:, :, 0:w]
                            pv = P[:sz, :].rearrange("p (h c) -> p h c", h=4)[:, :, 0:w]
                            nc.scalar.activation(pv, stv, Exp)
                        for h in range(4):
                            nc.tensor.matmul(
                                attnT_ps[32 * h:32 * h + 32, :w],
                                Vt[i][:sz, 32 * h:32 * h + 32],
                                P[:sz, 512 * h:512 * h + w],
                                start=start, stop=stop,
                                tile_position=(0, 32 * h),
                            )
                            nc.tensor.matmul(
                                den_ps[32 * h:32 * h + 1, :w],
                                ones_kt[:sz, :],
                                P[:sz, 512 * h:512 * h + w],
                                start=start, stop=stop,
                                tile_position=(0, 32 * h),
                            )
                        continue
                    for pair in range(2):
                        st = stps.tile([128, 1024], f32, name="st", tag="st")
                        for hh in range(2):
                            h = 2 * pair + hh
                            nc.tensor.matmul(
                                st[:sz, 512 * hh:512 * hh + w],
                                KTt[32 * h:32 * h + 32, o:o + sz],
                                QT[32 * h:32 * h + 32, c0:c1],
                                start=True, stop=True,
                                tile_position=(32 * h, 0),
                            )
                        P = psb.tile([128, 1024], bf16, name="P", tag="P")
                        if w == 512:
                            nc.scalar.activation(P[:sz, :], st[:sz, :], Exp)
                        else:
                            for hh in range(2):
                                nc.scalar.activation(
                                    P[:sz, 512 * hh:512 * hh + w],
                                    st[:sz, 512 * hh:512 * hh + w], Exp,
                                )
                        for hh in range(2):
                            h = 2 * pair + hh
                            nc.tensor.matmul(
                                attnT_ps[32 * h:32 * h + 32, :w],
                                Vt[i][:sz, 32 * h:32 * h + 32],
                                P[:sz, 512 * hh:512 * hh + w],
                                start=start, stop=stop,
                                tile_position=(0, 32 * h),
                            )
                            nc.tensor.matmul(
                                den_ps[32 * h:32 * h + 1, :w],
                                ones_kt[:sz, :],
                                P[:sz, 512 * hh:512 * hh + w],
                                start=start, stop=stop,
                                tile_position=(0, 32 * h),
                            )
                # normalization: recip of dens (rows 0,32,64,96; rest garbage)
                recip_sb = sb.tile([128, 512], f32, name="recip", tag="recip")
                nc.vector.reciprocal_approx_fast(recip_sb[0:97, :w], den_ps[0:97, :w])
                bcast_ps = tailps.tile([128, 512], f32, name="bcast", tag="tail")
                for h in range(4):
                    nc.tensor.matmul(
                        bcast_ps[32 * h:32 * h + 32, :w],
                        ones_col[32 * h:32 * h + 1, :],
                        recip_sb[32 * h:32 * h + 1, :w],
                        start=True, stop=True,
                        tile_position=(32 * h, 32 * h),
                    )
                bcast_sb = sb.tile([128, 512], f32, name="bcastsb", tag="bcastsb")
                nc.vector.tensor_copy(bcast_sb[:, :w], bcast_ps[:, :w])
                attn_sb = sb.tile([128, 512], f32r, name="attnsb", tag="attnsb")
                nc.vector.tensor_mul(attn_sb[:, :w], attnT_ps[:, :w], bcast_sb[:, :w])
                # out projection per 128-token subtile
                nsub = (w + 127) // 128
                for s4 in range(nsub):
                    ssz = min(128, w - 128 * s4)
                    off = 128 * s4
                    yps = tailps.tile([128, 512], f32, name="y", tag="tail")
                    nc.tensor.matmul(
                        yps[:ssz, 0:256],
                        attn_sb[:, off:off + ssz],
                        wout_sb[:],
                        start=True, stop=True,
                    )
                    ysb = sb.tile([128, 256], f32, name="ysb", tag="ysb")
                    nc.vector.tensor_copy(ysb[:ssz, :], yps[:ssz, 0:256])
                    nc.sync.dma_start(y_d[c0 + off:c0 + off + ssz, :], ysb[:ssz, :])

            if EXPW == 2048 and RUMP == "packed":
                # ---- rump chunk (q = 1536:1568, w=32): pack ALL 13 k-tiles'
                # scores into ONE 4-bank tile (head h bank: 13 blocks of 32),
                # exp them with a single strided ACT op ----
                c0, w = 1536, 32
                attnT_ps = avps.tile([128, 512], f32, name="attnTr", tag="attnT")
                den_ps = dnps.tile([128, 512], f32, name="denr", tag="den")
                stR = stps.tile([128, 2048], f32, name="stR", tag="st")
                for i, (o, sz) in enumerate(KTILES):
                    for h in range(4):
                        nc.tensor.matmul(
                            stR[:sz, 512 * h + 32 * i:512 * h + 32 * i + 32],
                            KTt[32 * h:32 * h + 32, o:o + sz],
                            QT[32 * h:32 * h + 32, c0:c0 + w],
                            start=True, stop=True,
                            tile_position=(32 * h, 0),
                        )
                PR = psb.tile([128, 2048], bf16, name="PR", tag="P")
                stv = stR[:, :].rearrange("p (h c) -> p h c", h=4)[:, :, 0:416]
                pv = PR[:, :].rearrange("p (h c) -> p h c", h=4)[:, :, 0:416]
                nc.scalar.activation(pv, stv, Exp)
                for i, (o, sz) in enumerate(KTILES):
                    start = (i == 0)
                    stop = (i == len(KTILES) - 1)
                    for h in range(4):
                        nc.tensor.matmul(
                            attnT_ps[32 * h:32 * h + 32, :w],
                            Vt[i][:sz, 32 * h:32 * h + 32],
                            PR[:sz, 512 * h + 32 * i:512 * h + 32 * i + 32],
                            start=start, stop=stop,
                            tile_position=(0, 32 * h),
                        )
                        nc.tensor.matmul(
                            den_ps[32 * h:32 * h + 1, :w],
                            ones_kt[:sz, :],
                            PR[:sz, 512 * h + 32 * i:512 * h + 32 * i + 32],
                            start=start, stop=stop,
                            tile_position=(0, 32 * h),
                        )
                recip_sb = sb.tile([128, 512], f32, name="recipr", tag="recip")
                nc.vector.reciprocal_approx_fast(recip_sb[0:97, :w], den_ps[0:97, :w])
                bcast_ps = tailps.tile([128, 512], f32, name="bcastr", tag="tail")
                for h in range(4):
                    nc.tensor.matmul(
                        bcast_ps[32 * h:32 * h + 32, :w],
                        ones_col[32 * h:32 * h + 1, :],
                        recip_sb[32 * h:32 * h + 1, :w],
                        start=True, stop=True,
                        tile_position=(32 * h, 32 * h),
                    )
                bcast_sb = sb.tile([128, 512], f32, name="bcastsbr", tag="bcastsb")
                nc.vector.tensor_copy(bcast_sb[:, :w], bcast_ps[:, :w])
                attn_sb = sb.tile([128, 512], f32r, name="attnsbr", tag="attnsb")
                nc.vector.tensor_mul(attn_sb[:, :w], attnT_ps[:, :w], bcast_sb[:, :w])
                yps = tailps.tile([128, 512], f32, name="yr", tag="tail")
                nc.tensor.matmul(
                    yps[:w, 0:256],
                    attn_sb[:, 0:w],
                    wout_sb[:],
                    start=True, stop=True,
                )
                ysb = sb.tile([128, 256], f32, name="ysbr", tag="ysb")
                nc.vector.tensor_copy(ysb[:w, :], yps[:w, 0:256])
                nc.sync.dma_start(y_d[c0:c0 + w, :], ysb[:w, :])
        ctx.close()

    nc.compile()
    return nc


def prepare_in_maps(x1, x2, pos_emb, w_qkv, b_qkv, w_out, b_out):
    x1 = np.asarray(x1, dtype=np.float32)
    x2 = np.asarray(x2, dtype=np.float32)
    pos = np.asarray(pos_emb, dtype=np.float32).reshape(C)
    w_qkv = np.asarray(w_qkv, dtype=np.float32)
    b_qkv = np.asarray(b_qkv, dtype=np.float32)
    w_out = np.asarray(w_out, dtype=np.float32)
    b_out = np.asarray(b_out, dtype=np.float32)

    scale = 1.0 / np.sqrt(np.float32(DH))
    b_eff = b_qkv + w_qkv @ pos
    wq = w_qkv[0:C] * scale
    bq = b_eff[0:C] * scale
    wk = w_qkv[C:2 * C]
    bk = b_eff[C:2 * C]
    wv = w_qkv[2 * C:3 * C]
    bv = b_eff[2 * C:3 * C]

    in_maps = []
    for core in range(8):
        b = core // 2
        g = core % 2
        gsl = slice(GC * g, GC * (g + 1))
        osl = slice(GC * (1 - g), GC * (2 - g))
        wqkT = np.concatenate([wq[gsl], wk[gsl]], axis=0).T.copy()     # [C, 256]
        wvT = wv[gsl].T.copy()                                         # [C, GC] own group only
        woutT = w_out[:, gsl].T.copy()                                 # [GC, 256]
        bqk = np.stack([bq[gsl], bk[gsl]])[:, :, None].copy()          # [2, 128, 1]
        in_maps.append({
            "x1b": np.ascontiguousarray(x1[b].reshape(C, N)),
            "x2b": np.ascontiguousarray(x2[b].reshape(C, N)),
            "wqkT": np.ascontiguousarray(wqkT),
            "wvT": np.ascontiguousarray(wvT),
            "woutT": np.ascontiguousarray(woutT),
            "bqk": np.ascontiguousarray(bqk),
        })
    # out1+out2 folds two tokens, each carrying b_out and the V-bias term
    y_const = 2.0 * (b_out + w_out @ bv)  # [C]
    return in_maps, y_const


def get_nc(repeat=1):
    key = (EXPW, RUMP, QKDT, repeat)
    if key not in _cache:
        _cache[key] = _build_nc(repeat)
    return _cache[key]


def assemble(per_core_y, y_const):
    out = np.empty((B, C, H, W), dtype=np.float32)
    for b in range(B):
        yb = per_core_y[2 * b] + per_core_y[2 * b + 1]                 # [S, C]
        yf = yb[:N] + yb[N:] + y_const[None, :]                        # [N, C]
        out[b] = yf.T.reshape(C, H, W)
    return out


def kernel(x1, x2, pos_emb, w_qkv, b_qkv, w_out, b_out):
    global LAST_RESULTS
    from concourse.bass_utils import run_bass_kernel_spmd

    in_maps, y_const = prepare_in_maps(x1, x2, pos_emb, w_qkv, b_qkv, w_out, b_out)
    nc = get_nc()
    res = run_bass_kernel_spmd(nc, in_maps, core_ids=list(range(8)))
    LAST_RESULTS = res
    return assemble([res.results[c]["y"] for c in range(8)], y_const)

